# revision 17
# baseline (speedup 1.0000x reference)
"""Trainium2 Bass kernel for nn_LongformerEncoder.

Sharding: 8 cores = (batch b in 0..3, seq-half p in 0..1).
Stage A (longformer layer) runs on 1024 own tokens (+256-token halo for the
sliding-window attention).  A pairwise AllGather exchanges the stage-A output
between the two cores of each batch; stage B (full 4-head/768-dim MHA +
max-pool) runs seq-split on queries with full keys/values, partial max per
core, final max across the pair on host.

All matmuls run in bf16 with f32 PSUM accumulation; layernorm stats/softmax
run in f32.  Embedding gather (emb[x] + pos) + mask/bias construction happen
on host as input preprocessing; per-core differences (sequence edges, key
masks) are pure input *data*, so a single SPMD program serves all 8 cores.
"""

import sys

sys.path.insert(0, "/opt/trn_rl_repo")

import numpy as np
import ml_dtypes

import concourse.bass as bass
import concourse.tile as tile
from concourse import bacc, mybir
from concourse.bass_utils import run_bass_kernel_spmd
from concourse.masks import make_identity

F32 = mybir.dt.float32
BF16 = mybir.dt.bfloat16
AX = mybir.AxisListType
ALU = mybir.AluOpType
ACTF = mybir.ActivationFunctionType

B, S, D = 4, 2048, 768
H_LF, DH = 12, 64
W = 256
DFF = 3072
NH, DK = 4, 768
T = 1024            # own tokens per core
EXT = 1536          # own + 256 halo each side
NEG = -1e9
EPS = 1e-5
NCORES = 8

# static (qtile -> list of (delta_idx, mask_slot)) for the LF band mask.
# mask slots: 0=TRI_A (kj>=qi), 1=TRI_B (kj<=qi), 2..7 = per-core e0..e5
LF_MASK_PLAN = {
    0: [(0, 2), (1, 3), (4, 1)],
    1: [(0, 4), (4, 1)],
    2: [(0, 0), (4, 1)],
    3: [(0, 0), (4, 1)],
    4: [(0, 0), (4, 1)],
    5: [(0, 0), (4, 1)],
    6: [(0, 0), (4, 5)],
    7: [(0, 0), (3, 6), (4, 7)],
}


def _bcast(handle, n, p=128):
    ap = handle.ap()
    return bass.AP(tensor=ap.tensor, offset=ap.offset, ap=[[0, p], [1, n]])


def build(debug=False):
    nc = bacc.Bacc("TRN2", target_bir_lowering=False, debug=False,
                   num_devices=NCORES)

    h0x_d = nc.dram_tensor("h0x", [EXT, D], F32, kind="ExternalInput")
    m640_d = nc.dram_tensor("m640", [8, 128, 640], F32, kind="ExternalInput")
    kbias_d = nc.dram_tensor("kbias", [S], F32, kind="ExternalInput")
    ln_d = {}
    for nm in ["lneg", "lneb", "ln1g", "ln1b", "ln2g", "ln2b", "mlng", "mlnb"]:
        ln_d[nm] = nc.dram_tensor(nm, [D], F32, kind="ExternalInput")
    lfw_d = {}
    for nm in ["lfwq", "lfwk", "lfwv", "lfwo"]:
        lfw_d[nm] = nc.dram_tensor(nm, [D, D], BF16, kind="ExternalInput")
    w1_d = nc.dram_tensor("w1", [D, DFF], BF16, kind="ExternalInput")
    w2_d = nc.dram_tensor("w2", [DFF, D], BF16, kind="ExternalInput")
    mw_d = {}
    for nm in ["mwq", "mwv"]:
        mw_d[nm] = nc.dram_tensor(nm, [D, NH * DK], BF16, kind="ExternalInput")
    mw_d["mwkT"] = nc.dram_tensor("mwkT", [NH * DK, D], BF16,
                                  kind="ExternalInput")
    mfc_d = nc.dram_tensor("mfc", [NH * DK, D], BF16, kind="ExternalInput")
    out_d = nc.dram_tensor("out", [128, 6], F32, kind="ExternalOutput")
    taps = {}
    if debug:
        taps["tap_olf"] = nc.dram_tensor("tap_olf", [T, D], F32,
                                         kind="ExternalOutput")
        taps["tap_attn"] = nc.dram_tensor("tap_attn", [T, D], F32,
                                          kind="ExternalOutput")

    with tile.TileContext(nc) as tc:
        _body(nc, tc, h0x_d, m640_d, kbias_d, ln_d, lfw_d, w1_d, w2_d,
              mw_d, mfc_d, out_d, taps)
    nc.compile()
    return nc


def _layernorm_tile(nc, pool, x_ap, g_b, b_b, out_tile, eps_ap):
    """out = (x - mean)/sqrt(var+eps) * g + b over free dim (768)."""
    stats = pool.tile([128, 2, 6], F32, tag="lnstats")
    nc.vector.bn_stats(out=stats[:, 0, :], in_=x_ap[:, 0:384])
    nc.vector.bn_stats(out=stats[:, 1, :], in_=x_ap[:, 384:768])
    mv = pool.tile([128, 2], F32, tag="lnmv")
    nc.vector.bn_aggr(out=mv, in_=stats)
    rstd = pool.tile([128, 1], F32, tag="lnrstd")
    nc.scalar.activation(out=rstd, in_=mv[:, 1:2], func=ACTF.Sqrt, bias=eps_ap)
    nc.vector.reciprocal(out=rstd, in_=rstd)
    nc.vector.tensor_scalar(out=out_tile, in0=x_ap, scalar1=mv[:, 0:1],
                            scalar2=rstd, op0=ALU.subtract, op1=ALU.mult)
    nc.vector.tensor_tensor(out_tile, out_tile, g_b, ALU.mult)
    nc.vector.tensor_tensor(out_tile, out_tile, b_b, ALU.add)


def _body(nc, tc, h0x_d, m640_d, kbias_d, ln_d, lfw_d, w1_d, w2_d,
          mw_d, mfc_d, out_d, taps):
    import contextlib
    ctx = contextlib.ExitStack()
    with ctx:
        constg = ctx.enter_context(tc.tile_pool(name="constg", bufs=1))
        dram = ctx.enter_context(tc.tile_pool(name="dram", bufs=1, space="DRAM"))

        id_bf = constg.tile([128, 128], BF16, tag="id_bf")
        make_identity(nc, id_bf)
        id_f32 = constg.tile([128, 128], F32, tag="id_f32")
        make_identity(nc, id_f32)
        mlng_b = constg.tile([128, D], BF16, tag="mlng")
        nc.gpsimd.dma_start(out=mlng_b, in_=_bcast(ln_d["mlng"], D))
        mlnb_b = constg.tile([128, D], BF16, tag="mlnb")
        nc.gpsimd.dma_start(out=mlnb_b, in_=_bcast(ln_d["mlnb"], D))
        eps_sb = constg.tile([128, 1], F32, tag="eps")
        nc.vector.memset(eps_sb, EPS)
        kbias_b = constg.tile([128, S], BF16, tag="kbias")
        nc.gpsimd.dma_start(out=kbias_b, in_=_bcast(kbias_d, S))

        # DRAM bounce for the collective
        src_olf = dram.tile([T, D], BF16)
        dst_olf = dram.tile([2 * T, D], BF16)

        # ============ STAGE A ============
        with tc.tile_pool(name="constA", bufs=1) as constA, \
             tc.tile_pool(name="attA", bufs=1) as attA, \
             tc.tile_pool(name="mid", bufs=1) as mid, \
             tc.tile_pool(name="lfw", bufs=2) as lfw, \
             tc.tile_pool(name="bigw", bufs=3) as bigw, \
             tc.tile_pool(name="work", bufs=2) as work, \
             tc.tile_pool(name="sm", bufs=4) as sm:

            lnA = {}
            for nm in ["lneg", "lneb", "ln1g", "ln1b", "ln2g", "ln2b"]:
                lnA[nm] = constA.tile([128, D], BF16, tag=nm, name=nm)
                nc.gpsimd.dma_start(out=lnA[nm], in_=_bcast(ln_d[nm], D))

            # ---- LN_e + transpose to feature-major hxT [128, 6, EXT] bf16
            ps1 = tc.tile_pool(name="ps1", bufs=2, space="PSUM")
            psG = ps1.__enter__()
            ps1b = tc.tile_pool(name="ps1b", bufs=2, space="PSUM")
            psT = ps1b.__enter__()
            hxT = attA.tile([128, 6, EXT], BF16, tag="hxT")
            for t in range(12):
                h0t = work.tile([128, D], F32, tag="h0t")
                nc.sync.dma_start(h0t, h0x_d.ap()[t * 128:(t + 1) * 128, :])
                hnb = work.tile([128, D], BF16, tag="hnb")
                _layernorm_tile(nc, sm, h0t, lnA["lneg"], lnA["lneb"], hnb, eps_sb)
                for c in range(6):
                    pt = psT.tile([128, 128], BF16, tag="tp")
                    nc.tensor.transpose(pt, hnb[:, c * 128:(c + 1) * 128], id_bf)
                    nc.vector.tensor_copy(out=hxT[:, c, t * 128:(t + 1) * 128],
                                          in_=pt)

            # ---- q/k feature-major, v token-major
            wq_sb = lfw.tile([128, 6, D], BF16, tag="lfw")
            nc.sync.dma_start(wq_sb, lfw_d["lfwq"].ap().rearrange(
                "(o p) f -> p o f", p=128))
            qT = attA.tile([128, 6, T], BF16, tag="qT")
            for f in range(6):
                for nch in range(2):
                    ps = psG.tile([128, 512], F32, tag="g")
                    for k in range(6):
                        nc.tensor.matmul(
                            ps, wq_sb[:, k, f * 128:(f + 1) * 128],
                            hxT[:, k, 256 + nch * 512: 256 + (nch + 1) * 512],
                            start=(k == 0), stop=(k == 5))
                    nc.any.tensor_copy(out=qT[:, f, nch * 512:(nch + 1) * 512],
                                       in_=ps)
            wk_sb = lfw.tile([128, 6, D], BF16, tag="lfw")
            nc.sync.dma_start(wk_sb, lfw_d["lfwk"].ap().rearrange(
                "(o p) f -> p o f", p=128))
            kT = attA.tile([128, 6, EXT], BF16, tag="kT")
            for f in range(6):
                for nch in range(3):
                    ps = psG.tile([128, 512], F32, tag="g")
                    for k in range(6):
                        nc.tensor.matmul(
                            ps, wk_sb[:, k, f * 128:(f + 1) * 128],
                            hxT[:, k, nch * 512:(nch + 1) * 512],
                            start=(k == 0), stop=(k == 5))
                    nc.any.tensor_copy(out=kT[:, f, nch * 512:(nch + 1) * 512],
                                       in_=ps)
            wv_sb = lfw.tile([128, 6, D], BF16, tag="lfw")
            nc.sync.dma_start(wv_sb, lfw_d["lfwv"].ap().rearrange(
                "(o p) f -> p o f", p=128))
            vtok = attA.tile([128, 12, D], BF16, tag="vtok")
            for t in range(12):
                for (n0, nn) in ((0, 512), (512, 256)):
                    ps = psG.tile([128, 512], F32, tag="g")
                    for k in range(6):
                        nc.tensor.matmul(
                            ps[:, :nn], hxT[:, k, t * 128:(t + 1) * 128],
                            wv_sb[:, k, n0:n0 + nn],
                            start=(k == 0), stop=(k == 5))
                    nc.any.tensor_copy(out=vtok[:, t, n0:n0 + nn],
                                       in_=ps[:, :nn])

            ps1b.__exit__(None, None, None)
            ps1.__exit__(None, None, None)

            # ---- sliding-window attention (12 heads as 6 pairs)
            ps2 = tc.tile_pool(name="ps2", bufs=1, space="PSUM")
            psS = ps2.__enter__()
            ps2b = tc.tile_pool(name="ps2b", bufs=2, space="PSUM")
            psT = ps2b.__enter__()
            ps2c = tc.tile_pool(name="ps2c", bufs=1, space="PSUM")
            psV = ps2c.__enter__()
            aT = attA.tile([128, 6, T], BF16, tag="aT")
            for qt in range(8):
                m640 = work.tile([128, 640], F32, tag="m640")
                nc.sync.dma_start(m640, m640_d.ap()[qt])
                for pair in range(6):
                    pss = []
                    for h2 in range(2):
                        ps = psS.tile([128, 640], F32, tag=f"sc{h2}",
                                      name=f"sc{h2}")
                        lhs = qT[h2 * 64:(h2 + 1) * 64, pair,
                                 qt * 128:(qt + 1) * 128]
                        nc.tensor.matmul(
                            ps[:, 0:512],
                            lhs, kT[h2 * 64:(h2 + 1) * 64, pair,
                                    qt * 128: qt * 128 + 512],
                            start=True, stop=True,
                            tile_position=(h2 * 64, 0))
                        nc.tensor.matmul(
                            ps[:, 512:640],
                            lhs, kT[h2 * 64:(h2 + 1) * 64, pair,
                                    qt * 128 + 512: qt * 128 + 640],
                            start=True, stop=True,
                            tile_position=(h2 * 64, 0))
                        pss.append(ps)
                    pvs = [psV.tile([128, 128], F32, tag=f"pv{i}",
                                    name=f"pv{i}") for i in range(2)]
                    for h2, ps in enumerate(pss):
                        sb = work.tile([128, 640], F32, tag=f"scsb{h2}",
                                       name=f"scsb{h2}")
                        nc.vector.tensor_tensor(sb, ps, m640, ALU.add)
                        mx = sm.tile([128, 1], F32, tag="mx")
                        nc.vector.tensor_reduce(out=mx, in_=sb, axis=AX.X,
                                                op=ALU.max)
                        nmx = sm.tile([128, 1], F32, tag="nmx")
                        nc.vector.tensor_scalar_mul(nmx, mx, -0.125)
                        probs = work.tile([128, 640], BF16, tag="probs")
                        sme = sm.tile([128, 1], F32, tag="sme")
                        nc.scalar.activation(out=probs, in_=sb, func=ACTF.Exp,
                                             bias=nmx, scale=0.125,
                                             accum_out=sme)
                        rs = sm.tile([128, 1], F32, tag="rs")
                        nc.vector.reciprocal(rs, sme)
                        nc.vector.tensor_scalar_mul(probs, probs, rs)
                        h = 2 * pair + h2
                        pt_sb = work.tile([128, 5, 128], BF16, tag="ptsb")
                        for dx in range(5):
                            ptp = psT.tile([128, 128], BF16, tag="tp")
                            nc.tensor.transpose(
                                ptp, probs[:, dx * 128:(dx + 1) * 128], id_bf)
                            nc.vector.tensor_copy(out=pt_sb[:, dx, :], in_=ptp)
                        pvt = pvs[h2]
                        for dx in range(5):
                            nc.tensor.matmul(
                                pvt[h2 * 64:(h2 + 1) * 64, :],
                                vtok[:, qt + dx, h * 64:(h + 1) * 64],
                                pt_sb[:, dx, :], start=(dx == 0),
                                stop=(dx == 4),
                                tile_position=(0, h2 * 64))
                    for h2 in range(2):
                        nc.any.tensor_copy(
                            out=aT[h2 * 64:(h2 + 1) * 64, pair,
                                   qt * 128:(qt + 1) * 128],
                            in_=pvs[h2][h2 * 64:(h2 + 1) * 64, :])

            ps2c.__exit__(None, None, None)
            ps2b.__exit__(None, None, None)
            ps2.__exit__(None, None, None)

            # ---- wo + residual (feature-major)
            ps3 = tc.tile_pool(name="ps3", bufs=2, space="PSUM")
            psG = ps3.__enter__()
            ps3b = tc.tile_pool(name="ps3b", bufs=2, space="PSUM")
            psT = ps3b.__enter__()
            wo_sb = lfw.tile([128, 6, D], BF16, tag="lfw")
            nc.sync.dma_start(wo_sb, lfw_d["lfwo"].ap().rearrange(
                "(o p) f -> p o f", p=128))
            r1T = mid.tile([128, 6, T], BF16, tag="resT")
            for f in range(6):
                for nch in range(2):
                    ps = psG.tile([128, 512], F32, tag="g")
                    for k in range(6):
                        nc.tensor.matmul(
                            ps, wo_sb[:, k, f * 128:(f + 1) * 128],
                            aT[:, k, nch * 512:(nch + 1) * 512],
                            start=(k == 0), stop=(k == 5))
                    nc.vector.tensor_tensor(
                        r1T[:, f, nch * 512:(nch + 1) * 512], ps,
                        hxT[:, f, 256 + nch * 512: 256 + (nch + 1) * 512],
                        ALU.add)

            # ---- LN1 (transpose to token-major, LN, transpose back)
            h1T = mid.tile([128, 6, T], BF16, tag="h1T")
            for t in range(8):
                rtok = work.tile([128, D], BF16, tag="rtok")
                for c in range(6):
                    pt = psT.tile([128, 128], BF16, tag="tp")
                    nc.tensor.transpose(pt, r1T[:, c, t * 128:(t + 1) * 128],
                                        id_bf)
                    nc.vector.tensor_copy(out=rtok[:, c * 128:(c + 1) * 128],
                                          in_=pt)
                ltok = work.tile([128, D], BF16, tag="ltok")
                _layernorm_tile(nc, sm, rtok, lnA["ln1g"], lnA["ln1b"], ltok, eps_sb)
                for c in range(6):
                    pt = psT.tile([128, 128], BF16, tag="tp")
                    nc.tensor.transpose(pt, ltok[:, c * 128:(c + 1) * 128],
                                        id_bf)
                    nc.vector.tensor_copy(out=h1T[:, c, t * 128:(t + 1) * 128],
                                          in_=pt)

            ps3b.__exit__(None, None, None)
            ps3.__exit__(None, None, None)

            # ---- FFN (streamed over dff chunks) + residual + LN2
            ps4 = tc.tile_pool(name="ps4", bufs=2, space="PSUM")
            psG = ps4.__enter__()
            ps4b = tc.tile_pool(name="ps4b", bufs=1, space="PSUM")
            psF = ps4b.__enter__()
            r2T = mid.tile([128, 6, T], BF16, tag="resT")
            for nch in range(2):
                f2ps = [psF.tile([128, 512], F32, tag=f"f2_{m}", name=f"f2_{m}")
                        for m in range(6)]
                for kc in range(24):
                    w1c = bigw.tile([128, 6, 128], BF16, tag="w1c")
                    nc.sync.dma_start(w1c, w1_d.ap()[
                        :, kc * 128:(kc + 1) * 128].rearrange(
                        "(o p) f -> p o f", p=128))
                    w2c = bigw.tile([128, D], BF16, tag="w2c")
                    nc.sync.dma_start(w2c, w2_d.ap()[
                        kc * 128:(kc + 1) * 128, :])
                    g1p = psG.tile([128, 512], F32, tag="g")
                    for k in range(6):
                        nc.tensor.matmul(
                            g1p, w1c[:, k, :],
                            h1T[:, k, nch * 512:(nch + 1) * 512],
                            start=(k == 0), stop=(k == 5))
                    g1c = work.tile([128, 512], BF16, tag="g1c")
                    nc.scalar.activation(out=g1c, in_=g1p,
                                         func=ACTF.Gelu_apprx_tanh)
                    for m in range(6):
                        nc.tensor.matmul(
                            f2ps[m], w2c[:, m * 128:(m + 1) * 128],
                            g1c, start=(kc == 0), stop=(kc == 23))
                for m in range(6):
                    nc.vector.tensor_tensor(
                        r2T[:, m, nch * 512:(nch + 1) * 512], f2ps[m],
                        h1T[:, m, nch * 512:(nch + 1) * 512], ALU.add)

            ps4b.__exit__(None, None, None)
            ps4.__exit__(None, None, None)
            ps5 = tc.tile_pool(name="ps5", bufs=2, space="PSUM")
            psT = ps5.__enter__()
            for t in range(8):
                rtok = work.tile([128, D], BF16, tag="rtok")
                for c in range(6):
                    pt = psT.tile([128, 128], BF16, tag="tp")
                    nc.tensor.transpose(pt, r2T[:, c, t * 128:(t + 1) * 128],
                                        id_bf)
                    nc.vector.tensor_copy(out=rtok[:, c * 128:(c + 1) * 128],
                                          in_=pt)
                otok = work.tile([128, D], BF16, tag="ltok")
                _layernorm_tile(nc, sm, rtok, lnA["ln2g"], lnA["ln2b"], otok, eps_sb)
                nc.sync.dma_start(src_olf[t * 128:(t + 1) * 128, :], otok)
                if "tap_olf" in taps:
                    of = work.tile([128, D], F32, tag="tapolf")
                    nc.vector.tensor_copy(out=of, in_=otok)
                    nc.sync.dma_start(
                        taps["tap_olf"].ap()[t * 128:(t + 1) * 128, :], of)

            ps5.__exit__(None, None, None)

        # ---- pairwise exchange of stage-A output
        nc.gpsimd.collective_compute(
            "AllGather", ALU.bypass,
            replica_groups=[[0, 1], [2, 3], [4, 5], [6, 7]],
            ins=[src_olf[:].opt()], outs=[dst_olf[:].opt()])

        # ============ STAGE B ============
        with tc.tile_pool(name="resB", bufs=1) as resB, \
             tc.tile_pool(name="whead", bufs=2) as whead, \
             tc.tile_pool(name="hb", bufs=1) as hb, \
             tc.tile_pool(name="q2pool", bufs=2) as q2pool, \
             tc.tile_pool(name="workB", bufs=2) as workB, \
             tc.tile_pool(name="smB", bufs=4) as smB, \
             tc.tile_pool(name="scB", bufs=1) as scB, \
             tc.tile_pool(name="psG2", bufs=2, space="PSUM") as psG2, \
             tc.tile_pool(name="psT2", bufs=2, space="PSUM") as psT2, \
             tc.tile_pool(name="psP2", bufs=1, space="PSUM") as psP2:

            # own-half feature-major copy (collective-independent)
            ownT = resB.tile([128, 6, T], BF16, tag="ownT")
            for c in range(6):
                nc.sync.dma_start_transpose(
                    ownT[:, c, :], src_olf[:, c * 128:(c + 1) * 128])

            def query_prep(h):
                """qTh = wq_h^T @ own; q2T = wk_h @ qTh (own/weights only)."""
                wqh = whead.tile([128, 6, DK], BF16, tag="wh", name="wqh")
                nc.sync.dma_start(wqh, mw_d["mwq"].ap()[
                    :, h * DK:(h + 1) * DK].rearrange("(o p) f -> p o f", p=128))
                qTh = hb.tile([128, 6, T], BF16, tag="qTh", name="qTh")
                for f in range(6):
                    for nch in range(2):
                        ps = psG2.tile([128, 512], F32, tag="g2", name="psq")
                        for k in range(6):
                            nc.tensor.matmul(
                                ps, wqh[:, k, f * 128:(f + 1) * 128],
                                ownT[:, k, nch * 512:(nch + 1) * 512],
                                start=(k == 0), stop=(k == 5))
                        nc.any.tensor_copy(
                            out=qTh[:, f, nch * 512:(nch + 1) * 512], in_=ps)
                wkT = hb.tile([128, 6, D], BF16, tag="wkT", name="wkT")
                nc.sync.dma_start(wkT, mw_d["mwkT"].ap()[
                    h * DK:(h + 1) * DK, :].rearrange("(o p) f -> p o f", p=128))
                q2T = q2pool.tile([128, 6, T], BF16, tag="q2T", name="q2T")
                for f in range(6):
                    for nch in range(2):
                        ps = psG2.tile([128, 512], F32, tag="g2", name="psq2")
                        for k in range(6):
                            nc.tensor.matmul(
                                ps, wkT[:, k, f * 128:(f + 1) * 128],
                                qTh[:, k, nch * 512:(nch + 1) * 512],
                                start=(k == 0), stop=(k == 5))
                        nc.any.tensor_copy(
                            out=q2T[:, f, nch * 512:(nch + 1) * 512], in_=ps)
                return q2T

            q2T_pre = [query_prep(0), query_prep(1)]

            # gathered full sequence, feature-major (needs the collective)
            olfT = resB.tile([128, 6, S], BF16, tag="olfT")
            for c in range(6):
                nc.sync.dma_start_transpose(
                    olfT[:, c, :], dst_olf[:, c * 128:(c + 1) * 128])

            # token-major copy of gathered sequence (PV rhs)
            olftok = resB.tile([128, 16, D], BF16, tag="olftok")
            for t in range(16):
                nc.sync.dma_start(olftok[:, t, :],
                                  dst_olf[t * 128:(t + 1) * 128, :])

            fcacc = resB.tile([128, 6, T], BF16, tag="fcacc")
            for h in range(NH):
                if h < 2:
                    q2T = q2T_pre[h]
                else:
                    q2T = query_prep(h)
                wvh = whead.tile([128, 6, DK], BF16, tag="wh")
                nc.sync.dma_start(wvh, mw_d["mwv"].ap()[
                    :, h * DK:(h + 1) * DK].rearrange("(o p) f -> p o f", p=128))
                attnTh = hb.tile([128, 6, T], BF16, tag="attnTh")
                poT_all = hb.tile([128, 6, T], BF16, tag="qTh", name="poT_all")
                for qt in range(8):
                    # scores[q, k] = q2T^T · olfT  (contraction over din)
                    ssb = scB.tile([128, S], F32, tag="ssb")
                    for kc in range(4):
                        ps = psG2.tile([128, 512], F32, tag="g2")
                        for k in range(6):
                            nc.tensor.matmul(
                                ps, q2T[:, k, qt * 128:(qt + 1) * 128],
                                olfT[:, k, kc * 512:(kc + 1) * 512],
                                start=(k == 0), stop=(k == 5))
                        nc.vector.tensor_tensor(
                            ssb[:, kc * 512:(kc + 1) * 512], ps,
                            kbias_b[:, kc * 512:(kc + 1) * 512], ALU.add)
                    mxs = smB.tile([128, 4], F32, tag="mxs")
                    for kc in range(4):
                        nc.vector.tensor_reduce(
                            out=mxs[:, kc:kc + 1],
                            in_=ssb[:, kc * 512:(kc + 1) * 512],
                            axis=AX.X, op=ALU.max)
                    nmx = smB.tile([128, 1], F32, tag="nmx2")
                    nc.vector.tensor_reduce(out=nmx, in_=mxs, axis=AX.X,
                                            op=ALU.max)
                    nc.vector.tensor_scalar_mul(nmx, nmx, -1.0 / np.sqrt(DK))
                    probs = workB.tile([128, S], BF16, tag="probs2")
                    smes = smB.tile([128, 4], F32, tag="smes")
                    for kc in range(4):
                        nc.scalar.activation(
                            out=probs[:, kc * 512:(kc + 1) * 512],
                            in_=ssb[:, kc * 512:(kc + 1) * 512], func=ACTF.Exp,
                            bias=nmx, scale=1.0 / np.sqrt(DK),
                            accum_out=smes[:, kc:kc + 1])
                    sme = smB.tile([128, 1], F32, tag="sme2")
                    nc.vector.tensor_reduce(out=sme, in_=smes, axis=AX.X,
                                            op=ALU.add)
                    rs = smB.tile([128, 1], F32, tag="rs2")
                    nc.vector.reciprocal(rs, sme)
                    # po[q, din] = probs @ olf  (token-major rhs)
                    pt_sb = workB.tile([128, 16, 128], BF16, tag="ptsb2")
                    for kc in range(16):
                        ptp = psT2.tile([128, 128], BF16, tag="tp2")
                        nc.tensor.transpose(
                            ptp, probs[:, kc * 128:(kc + 1) * 128], id_bf)
                        nc.vector.tensor_copy(out=pt_sb[:, kc, :], in_=ptp)
                    pvp = psP2.tile([128, 768], F32, tag="pv2")
                    for kc in range(16):
                        for (n0, nn) in ((0, 512), (512, 256)):
                            nc.tensor.matmul(
                                pvp[:, n0:n0 + nn], pt_sb[:, kc, :],
                                olftok[:, kc, n0:n0 + nn],
                                start=(kc == 0), stop=(kc == 15))
                    po = workB.tile([128, D], BF16, tag="po")
                    nc.vector.tensor_scalar_mul(po, pvp, rs)
                    for c in range(6):
                        pt = psT2.tile([128, 128], BF16, tag="tp2")
                        nc.tensor.transpose(pt, po[:, c * 128:(c + 1) * 128],
                                            id_bf)
                        nc.vector.tensor_copy(
                            out=poT_all[:, c, qt * 128:(qt + 1) * 128], in_=pt)
                # attn_h^T = wv_h^T @ po^T  [dk, own], batched over q-tiles
                for m in range(6):
                    for nch in range(2):
                        ps = psG2.tile([128, 512], F32, tag="g2")
                        for k in range(6):
                            nc.tensor.matmul(
                                ps, wvh[:, k, m * 128:(m + 1) * 128],
                                poT_all[:, k, nch * 512:(nch + 1) * 512],
                                start=(k == 0), stop=(k == 5))
                        nc.any.tensor_copy(
                            out=attnTh[:, m, nch * 512:(nch + 1) * 512], in_=ps)

                fch = whead.tile([128, 6, DK], BF16, tag="wh")
                nc.sync.dma_start(fch, mfc_d.ap()[
                    h * DK:(h + 1) * DK, :].rearrange("(o p) f -> p o f", p=128))
                for m in range(6):
                    for nch in range(2):
                        ps = psG2.tile([128, 512], F32, tag="g2")
                        for k in range(6):
                            nc.tensor.matmul(
                                ps, fch[:, k, m * 128:(m + 1) * 128],
                                attnTh[:, k, nch * 512:(nch + 1) * 512],
                                start=(k == 0), stop=(k == 5))
                        dst = fcacc[:, m, nch * 512:(nch + 1) * 512]
                        if h == 0:
                            nc.vector.tensor_copy(out=dst, in_=ps)
                        else:
                            nc.vector.tensor_tensor(dst, dst, ps, ALU.add)

            # residual + LN + running max over own tokens
            maxacc = resB.tile([128, D], F32, tag="maxacc")
            for m in range(6):
                nc.vector.tensor_tensor(fcacc[:, m, :], fcacc[:, m, :],
                                        ownT[:, m, :], ALU.add)
            for t in range(8):
                rtok = workB.tile([128, D], BF16, tag="rtokB")
                for c in range(6):
                    pt = psT2.tile([128, 128], BF16, tag="tp2")
                    nc.tensor.transpose(pt, fcacc[:, c, t * 128:(t + 1) * 128],
                                        id_bf)
                    nc.vector.tensor_copy(out=rtok[:, c * 128:(c + 1) * 128],
                                          in_=pt)
                ltok = workB.tile([128, D], F32, tag="ltokB")
                _layernorm_tile(nc, smB, rtok, mlng_b, mlnb_b, ltok, eps_sb)
                if "tap_attn" in taps:
                    nc.sync.dma_start(
                        taps["tap_attn"].ap()[t * 128:(t + 1) * 128, :], ltok)
                if t == 0:
                    nc.vector.tensor_copy(out=maxacc, in_=ltok)
                else:
                    nc.vector.tensor_tensor(maxacc, maxacc, ltok, ALU.max)
            outsb = resB.tile([128, 6], F32, tag="outsb")
            for c in range(6):
                pt = psT2.tile([128, 128], F32, tag="tpf")
                nc.tensor.transpose(pt, maxacc[:, c * 128:(c + 1) * 128],
                                    id_f32)
                nc.vector.tensor_reduce(out=outsb[:, c:c + 1], in_=pt,
                                        axis=AX.X, op=ALU.max)
            nc.sync.dma_start(out_d.ap(), outsb)


# ---------------- host side ----------------

_NC_CACHE = {}


def _get_nc(debug=False):
    key = bool(debug)
    if key not in _NC_CACHE:
        _NC_CACHE[key] = build(debug=debug)
    return _NC_CACHE[key]


def _tri(cond):
    qi = np.arange(128)[:, None]
    kj = np.arange(128)[None, :]
    return np.where(cond(qi, kj), 0.0, NEG).astype(np.float32)


def _prep_in_maps(inputs):
    x = np.asarray(inputs["x"])
    emb = np.asarray(inputs["emb"], np.float32)
    pos = np.asarray(inputs["pos"], np.float32)
    bf = ml_dtypes.bfloat16
    wts = {
        "lfwq": inputs["lf_wq"], "lfwk": inputs["lf_wk"],
        "lfwv": inputs["lf_wv"], "lfwo": inputs["lf_wo"],
        "w1": inputs["w1"], "w2": inputs["w2"],
        "mwq": inputs["mha_wq"],
        "mwkT": np.ascontiguousarray(np.asarray(inputs["mha_wk"]).T),
        "mwv": inputs["mha_wv"], "mfc": inputs["mha_fc"],
    }
    wts = {k: np.ascontiguousarray(np.asarray(v, np.float32)).astype(bf)
           for k, v in wts.items()}
    lns = {
        "lneg": inputs["ln_e_g"], "lneb": inputs["ln_e_b"],
        "ln1g": inputs["ln1_g"], "ln1b": inputs["ln1_b"],
        "ln2g": inputs["ln2_g"], "ln2b": inputs["ln2_b"],
        "mlng": inputs["mha_ln_g"], "mlnb": inputs["mha_ln_b"],
    }
    lns = {k: np.ascontiguousarray(np.asarray(v, np.float32))
           for k, v in lns.items()}

    in_maps = []
    for c in range(NCORES):
        b, p = c // 2, c % 2
        h0 = emb[x[b]] + pos                       # [S, D] f32
        start = p * T - 256
        h0x = np.zeros((EXT, D), np.float32)
        lo, hi = max(0, start), min(S, start + EXT)
        h0x[lo - start: hi - start] = h0[lo:hi]

        # exact LF band+validity bias per q-tile: [8, 128, 640]
        qi = np.arange(128)
        kj = np.arange(640)
        m640 = np.zeros((8, 128, 640), np.float32)
        for qt in range(8):
            qg = p * T + qt * 128 + qi[:, None]
            kg = start + qt * 128 + kj[None, :]
            ok = (np.abs(kg - qg) <= W) & (kg >= 0) & (kg < S)
            m640[qt] = np.where(ok, 0.0, NEG)

        kbias = np.where(x[b] != 0, 0.0, NEG).astype(np.float32)

        m = {"h0x": h0x, "m640": m640, "kbias": kbias}
        m.update(lns)
        m.update(wts)
        in_maps.append(m)
    return in_maps


def _postprocess(results):
    out = np.zeros((B, D), np.float32)
    for b in range(B):
        m0 = np.asarray(results[2 * b]["out"]).T.reshape(D)
        m1 = np.asarray(results[2 * b + 1]["out"]).T.reshape(D)
        out[b] = np.maximum(m0, m1)
    return out


def run(inputs, debug=False, trace=False):
    nc = _get_nc(debug=debug)
    in_maps = _prep_in_maps(inputs)
    res = run_bass_kernel_spmd(nc, in_maps, core_ids=list(range(NCORES)),
                               trace=trace)
    return res


def kernel(**inputs):
    res = run(inputs, debug=False, trace=False)
    return _postprocess(res.results)
